# revision 6
# baseline (speedup 1.0000x reference)
"""AmpNorm Trainium2 kernel: FFT-domain amplitude normalization.

reference semantics:
    fft = fft2(x); amp = fftshift(|fft|); pha = angle(fft)
    amp_mean = mean(amp, axis=0)
    new_amp = (1-m)*running_amp + m*amp_mean     (EMA branch; init branch if sum==0)
    out = real(ifft2(ifftshift(new_amp) * exp(i*pha)))

Device formulation (per [512,512] image; shifts absorbed on host):
    Z = F @ X @ F, computed only for column frequencies k_c in [0, 256]
    (x real => Z Hermitian; the ratio is symmetrized host-side so the
    half-spectrum determines the output exactly).
    amp = |Z|; per-channel amp_sum AllReduced over the 8 cores.
    s = (ra_sym + mom*amp_sum_red) / amp        (symmetric real ratio)
    W = Z * s
    T2 = G @ W   (inverse row transform, G = conj(F))
    out[n_r,n_c] = sum_{k_c=0}^{256} w_k * Re(T2[n_r,k_c] G[k_c,n_c]) / N^2
      with w = [1, 2...2, 1] -- folded into the Gw constants.

Row transforms use a radix-2 split: stage 1 is a DIF butterfly on the 512
input rows followed by 256-point matmuls against the even/odd columns of F
(so the row-frequency axis lives in [evens, odds] permuted order, which the
host bakes into ra); stage 3 is the matching DIT inverse (even/odd G
columns) whose output butterfly is fused into the PSUM evacuation.

All matmuls f32r (~bf16 speed, ~1.5e-4 rel err). Data-stationary form:
out = lhsT.T @ rhs with lhsT = data chunk, rhs = constant block; each stage
flips the layout, so no transposes. Z and 1/amp round-trip DRAM in bf16 as
one fused 3-plane tensor per image (single DMA each way). 1/amp comes from
one Abs_reciprocal_sqrt table lookup (amp = a2 * invamp on Pool) -- the
whole kernel runs on a single activation-table set. The per-channel
amp_sum AllReduce runs in bf16 (half the collective bytes). All DFT
constants (and mom) ride in one row-packed [R, 512] external tensor so a
kernel call carries 3 args instead of 16 (axon per-call dispatch cost
scales with arg count). Sharding: batch over 8 cores; amp mean via
per-channel AllReduce.
"""
import sys

sys.path.insert(0, "/opt/trn_rl_repo")

import numpy as np

N_CORES = 8
B, C, H, W = 32, 3, 512, 512
B_LOC = B // N_CORES          # 4 batches per core
N_IMG = B_LOC * C             # 12 images per core
NBLK = H // 128               # 4 partition blocks
KC = 264                      # half-spectrum cols: 257 used + 7 zero pad
MOMENTUM = 0.1

_cached = {}


def _build():
    from concourse import bacc, tile, mybir

    f32 = mybir.dt.float32
    f32r = mybir.dt.float32r
    bf16 = mybir.dt.bfloat16
    f16 = mybir.dt.float16
    Alu = mybir.AluOpType
    Act = mybir.ActivationFunctionType

    # Force every activation into the one table set covering
    # {copy, identity, square, abs_reciprocal_sqrt}: exactly one ACT table
    # load (the default per-function chooser thrashes sets, ~2.7us a reload).
    # Index order must be preserved, so other sets are stripped, not removed.
    from concourse import hw_specs as _hw
    if not getattr(_hw, "_ampnorm_patched", False):
        _orig_get_tables = _hw.get_activation_tables

        def _patched(module_arch):
            tabs = _orig_get_tables(module_arch)
            keep = "abs_reciprocal_sqrt_and_small"
            covered = tabs[keep]
            return {
                name: (fns if name == keep else (fns - covered))
                for name, fns in tabs.items()
            }

        _hw.get_activation_tables = _patched
        _hw._ampnorm_patched = True
        import concourse.bacc as _bacc_mod
        _bacc_mod.get_activation_tables = _patched

    nc = bacc.Bacc("TRN2", target_bir_lowering=False, debug=False,
                   num_devices=N_CORES)

    x_ext = nc.dram_tensor("x", [B_LOC, C, H, W], f32, kind="ExternalInput").ap()
    ra_ext = nc.dram_tensor("ra", [C, H, KC], f32, kind="ExternalInput").ap()
    cdefs = {
        # stage1: B0 = F[0:256, 0::2], B1 = F[0:256, 1::2]  (radix-2 DIF)
        "B0r": [256, 256], "B0i": [256, 256], "B1r": [256, 256], "B1i": [256, 256],
        # stage2 rhs: half-width F
        "Frh": [H, KC], "Fih": [H, KC], "negFih": [H, KC],
        # stage3: Me/Mo = even/odd columns of G = conj(F), concatenated
        # [real | imag] so one matmul produces both halves (radix-2 DIT)
        "MeRI": [256, 512], "MeIR": [256, 512],
        "MoRI": [256, 512], "MoIR": [256, 512],
        # stage4: symmetry-weighted inverse column matrices (incl 1/N^2)
        "Gw1": [KC, W], "Gw2": [KC, W],
    }
    # all constants (and mom, final row) ride in ONE row-concatenated
    # [R, 512] tensor: fewer executable args = less per-call dispatch cost
    crows = {}
    _r = 0
    for _n, _s in cdefs.items():
        crows[_n] = _r
        _r += _s[0]
    CPACK_ROWS = _r + 1          # +1 row for mom
    cpack_ext = nc.dram_tensor("cpack", [CPACK_ROWS, 512], f32,
                               kind="ExternalInput").ap()
    out_ext = nc.dram_tensor("out", [B_LOC, C, H, W], f32, kind="ExternalOutput").ap()

    q_slices = [(0, 128), (128, 256), (256, KC)]  # k_c chunks (2 full + slab)

    with tile.TileContext(nc) as tc:
        with (
            tc.tile_pool(name="const", bufs=1) as constp,
            tc.tile_pool(name="accum", bufs=1) as accp,
            tc.tile_pool(name="stage", bufs=1) as stagep,
            tc.tile_pool(name="work", bufs=2) as workp,
            tc.tile_pool(name="work3", bufs=3) as work3p,
            tc.tile_pool(name="stg2", bufs=2) as stg2p,
            tc.tile_pool(name="psum", bufs=3, space="PSUM") as psump,
            tc.tile_pool(name="psum1", bufs=2, space="PSUM") as psump1,
            tc.tile_pool(name="dram", bufs=1, space="DRAM") as dramp,
        ):
            def blocked(ap):  # [m*128+p, j] dram view -> [p, m, j]
                return ap.rearrange("(m p) j -> p m j", p=128)

            prefetched = {}

            # ---- constants: DMA f32, round to f32r on gpsimd ----
            # pass-1 consts load first; pass-2-only consts (Me*/Mo*/Gw*) are
            # deferred until after channel 0 is emitted so they don't delay
            # the first images' x loads in the DMA queue.
            cst = {}

            def load_consts(names):
                for name in names:
                    shp = cdefs[name]
                    r0 = crows[name]
                    tiles = []
                    nb = (shp[0] + 127) // 128
                    for k in range(nb):
                        p0 = k * 128
                        p1 = min(shp[0], p0 + 128)
                        stg = workp.tile([p1 - p0, shp[1]], f32,
                                         name=f"cstg_{name}_{k}", tag="cstg")
                        nc.sync.dma_start(
                            stg[:], cpack_ext[r0 + p0:r0 + p1, 0:shp[1]])
                        t = constp.tile([p1 - p0, shp[1]], f16,
                                        name=f"c_{name}_{k}")
                        nc.gpsimd.tensor_copy(t[:], stg[:])
                        tiles.append(t)
                    cst[name] = tiles

            p1_consts = ["B0r", "B0i", "B1r", "B1i", "Frh", "Fih", "negFih"]
            load_consts(["B0r", "B0i", "B1r", "B1i"])
            for pb in (0, 1):
                pt = workp.tile([128, NBLK, W], f32, name="xstg", tag="xstg")
                nc.sync.dma_start(pt[:], blocked(x_ext[pb, 0]))
                prefetched[(pb, 0)] = pt
            load_consts(["Frh", "Fih", "negFih"])

            mom_t = constp.tile([128, 1], f32, name="mom_t")
            nc.sync.dma_start(
                mom_t[:],
                cpack_ext[CPACK_ROWS - 1:CPACK_ROWS, 0:128].rearrange(
                    "o p -> p o"))

            # ---- per-channel amp_sum accumulators [128, 4, KC] ----
            amp_sum = {}
            for c in range(C):
                t = accp.tile([128, NBLK, KC], f32, name=f"asum_{c}")
                nc.gpsimd.memset(t[:], 0.0)
                amp_sum[c] = t

            # ---- DRAM scratch (bf16) + collective bounces (bf16) ----
            zscr = dramp.tile([N_IMG, 3, H, KC], f16, name="zscr")
            ar_in = [dramp.tile([H, KC], bf16, name=f"ar_in_{c}") for c in range(C)]
            ar_out = [
                dramp.tile([H, KC], bf16, name=f"ar_out_{c}", addr_space="Shared")
                for c in range(C)
            ]

            # ===== PASS 1 (per channel): forward + amp accumulation =====
            rats = {}

            def emit_p1(c):
                for b in range(B_LOC):
                    img = b * C + c
                    xstg = prefetched.pop((b, c), None)
                    if xstg is None:
                        xstg = workp.tile([128, NBLK, W], f32, name="xstg",
                                          tag="xstg")
                        nc.sync.dma_start(xstg[:], blocked(x_ext[b, c]))
                    # radix-2 DIF butterfly over rows (writes f32r directly)
                    y = {}
                    for (nm, j, op) in (("y0a", 0, Alu.add), ("y0b", 1, Alu.add),
                                        ("y1a", 0, Alu.subtract), ("y1b", 1, Alu.subtract)):
                        t = work3p.tile([128, W], f16, name=nm, tag=nm)
                        nc.vector.tensor_tensor(t[:], xstg[:, j, :], xstg[:, j + 2, :], op)
                        y[nm] = t
                    # stage 1: U^T[:, evens] = y0 @ B0; U^T[:, odds] = y1 @ B1
                    ur, ui = [], []
                    for m in range(NBLK):
                        ms = slice(m * 128, (m + 1) * 128)
                        psr = psump.tile([128, W], f32, name="ps1r", tag="psr")
                        psi = psump.tile([128, W], f32, name="ps1i", tag="psi")
                        for half, ya, yb, br, bi in (
                            (slice(0, 256), y["y0a"], y["y0b"], "B0r", "B0i"),
                            (slice(256, 512), y["y1a"], y["y1b"], "B1r", "B1i"),
                        ):
                            nc.tensor.matmul(psr[:, half], ya[:, ms], cst[br][0][:],
                                             start=True, stop=False)
                            nc.tensor.matmul(psr[:, half], yb[:, ms], cst[br][1][:],
                                             start=False, stop=True)
                            nc.tensor.matmul(psi[:, half], ya[:, ms], cst[bi][0][:],
                                             start=True, stop=False)
                            nc.tensor.matmul(psi[:, half], yb[:, ms], cst[bi][1][:],
                                             start=False, stop=True)
                        tr = stg2p.tile([128, W], f16, name=f"ur_{m}", tag=f"sr_{m}")
                        ti = stg2p.tile([128, W], f16, name=f"ui_{m}", tag=f"si_{m}")
                        nc.scalar.copy(tr[:], psr[:])
                        nc.vector.tensor_copy(ti[:], psi[:])
                        ur.append(tr)
                        ui.append(ti)
                    # stage 2: Z = U @ F_half  [k_r-part (permuted), k_c)
                    # zall planes: 0 = Z_re, 1 = Z_im, 2 = 1/|Z| (one DMA)
                    zall = workp.tile([128, 3, NBLK, KC], f16, name="zall", tag="zall")
                    for m in range(NBLK):
                        ms = slice(m * 128, (m + 1) * 128)
                        psr = psump.tile([128, KC], f32, name="ps2r", tag="psr")
                        psi = psump.tile([128, KC], f32, name="ps2i", tag="psi")
                        for k in range(NBLK):
                            nc.tensor.matmul(psr[:], ur[k][:, ms], cst["Frh"][k][:],
                                             start=(k == 0), stop=False)
                        for k in range(NBLK):
                            nc.tensor.matmul(psr[:], ui[k][:, ms], cst["negFih"][k][:],
                                             start=False, stop=(k == NBLK - 1))
                        for k in range(NBLK):
                            nc.tensor.matmul(psi[:], ur[k][:, ms], cst["Fih"][k][:],
                                             start=(k == 0), stop=False)
                        for k in range(NBLK):
                            nc.tensor.matmul(psi[:], ui[k][:, ms], cst["Frh"][k][:],
                                             start=False, stop=(k == NBLK - 1))
                        nc.vector.tensor_copy(zall[:, 0, m, :], psr[:])
                        nc.vector.tensor_copy(zall[:, 1, m, :], psi[:])
                        sq1 = workp.tile([128, KC], f32, name="sq1", tag="sq1")
                        sq2 = workp.tile([128, KC], f32, name="sq2", tag="sq2")
                        nc.scalar.square(sq1[:], psr[:])
                        nc.scalar.square(sq2[:], psi[:])
                        a2 = workp.tile([128, KC], f32, name="a2", tag="a2")
                        nc.vector.scalar_tensor_tensor(
                            a2[:], sq1[:], 1e-6, sq2[:],
                            op0=Alu.add, op1=Alu.add)
                        # 1/amp via one table lookup; amp = a2 * (1/amp) on Pool
                        nc.scalar.activation(zall[:, 2, m, :], a2[:],
                                             Act.Abs_reciprocal_sqrt)
                        am = workp.tile([128, KC], f32, name="am", tag="am")
                        nc.gpsimd.tensor_tensor(am[:], a2[:], zall[:, 2, m, :],
                                                Alu.mult)
                        nc.gpsimd.tensor_add(amp_sum[c][:, m, :], amp_sum[c][:, m, :],
                                             am[:])
                    nc.sync.dma_start(
                        zscr[img].rearrange("t (m p) j -> p t m j", p=128), zall[:])
                if c + 1 < C:
                    pt = workp.tile([128, NBLK, W], f32, name="xstg", tag="xstg")
                    nc.sync.dma_start(pt[:], blocked(x_ext[0, c + 1]))
                    prefetched[(0, c + 1)] = pt
                asum16 = workp.tile([128, NBLK, KC], bf16, name="asum16", tag="asum16")
                nc.gpsimd.tensor_copy(asum16[:], amp_sum[c][:])
                nc.sync.dma_start(blocked(ar_in[c][:, :]), asum16[:])
                nc.gpsimd.collective_compute(
                    "AllReduce",
                    Alu.add,
                    replica_groups=[list(range(N_CORES))],
                    ins=[ar_in[c].opt()],
                    outs=[ar_out[c].opt()],
                )
                rat = stagep.tile([128, NBLK, KC], f32, name=f"rat_{c}",
                                  tag=f"rat_{c}")
                nc.sync.dma_start(rat[:], blocked(ra_ext[c]))
                rats[c] = rat

            # ===== PASS 2 (per channel): ratio + inverse =====
            # ar_wait[c]: virtual-time floor (ms) keeping the collective's
            # consumers out of the engine/DMA queues until the AllReduce is
            # done -- otherwise they head-of-line-block the whole pipeline.
            ar_wait = {0: 0.145, 1: 0.200, 2: 0.255}

            def emit_p2(c):
                with tc.tile_wait_until(ar_wait[c]):
                    red = workp.tile([128, NBLK, KC], bf16, name="red", tag="red")
                    nc.sync.dma_start(red[:], blocked(ar_out[c][:, :]))
                    numer = amp_sum[c]
                    nc.vector.scalar_tensor_tensor(
                        numer[:], red[:], mom_t[:, 0:1], rats[c][:],
                        op0=Alu.mult, op1=Alu.add)
                for b in range(B_LOC):
                    img = b * C + c
                    zall_l = workp.tile([128, 3, NBLK, KC], f16, name="zall_l",
                                        tag="zall")
                    nc.sync.dma_start(
                        zall_l[:], zscr[img].rearrange("t (m p) j -> p t m j", p=128))
                    twr, twi = [], []
                    for m in range(NBLK):
                        rn = workp.tile([128, KC], f32, name="rn", tag="rn")
                        nc.vector.tensor_mul(rn[:], numer[:, m, :], zall_l[:, 2, m, :])
                        wr_t = stg2p.tile([128, W], f16, name=f"wr_{m}", tag=f"sr_{m}")
                        wi_t = stg2p.tile([128, W], f16, name=f"wi_{m}", tag=f"si_{m}")
                        nc.vector.tensor_mul(wr_t[:, 0:KC], zall_l[:, 0, m, :], rn[:])
                        nc.vector.tensor_mul(wi_t[:, 0:KC], zall_l[:, 1, m, :], rn[:])
                        twr.append(wr_t)
                        twi.append(wi_t)
                    # stage 3 (radix-2 DIT over permuted k_r):
                    #   E = W_even^T Me, O = W_odd^T Mo;  T2^T = [E+O | E-O]
                    t2r, t2i = [], []
                    for qi, (q0, q1) in enumerate(q_slices):
                        qs = slice(q0, q1)
                        qn = q1 - q0
                        psE = psump.tile([qn, 512], f32, name="psE", tag="psr")
                        psO = psump.tile([qn, 512], f32, name="psO", tag="psi")
                        for k in range(2):
                            nc.tensor.matmul(psE[:], twr[k][:, qs], cst["MeRI"][k][:],
                                             start=(k == 0), stop=False)
                        for k in range(2):
                            nc.tensor.matmul(psE[:], twi[k][:, qs], cst["MeIR"][k][:],
                                             start=False, stop=(k == 1))
                        for k in range(2):
                            nc.tensor.matmul(psO[:], twr[k + 2][:, qs], cst["MoRI"][k][:],
                                             start=(k == 0), stop=False)
                        for k in range(2):
                            nc.tensor.matmul(psO[:], twi[k + 2][:, qs], cst["MoIR"][k][:],
                                             start=False, stop=(k == 1))
                        er = workp.tile([qn, 256], f32r, name="er", tag="er")
                        ei = workp.tile([qn, 256], f32r, name="ei", tag="ei")
                        nc.scalar.copy(er[:], psE[:, 0:256])
                        nc.scalar.copy(ei[:], psE[:, 256:512])
                        t2p = stg2p if qi < 2 else stagep
                        rt = t2p.tile([qn, 256], f16, name=f"t2rt_{qi}", tag=f"t2rt_{qi}")
                        rb = t2p.tile([qn, 256], f16, name=f"t2rb_{qi}", tag=f"t2rb_{qi}")
                        it_ = t2p.tile([qn, 256], f16, name=f"t2it_{qi}", tag=f"t2it_{qi}")
                        ib = t2p.tile([qn, 256], f16, name=f"t2ib_{qi}", tag=f"t2ib_{qi}")
                        nc.vector.tensor_add(rt[:], er[:], psO[:, 0:256])
                        nc.vector.tensor_sub(rb[:], er[:], psO[:, 0:256])
                        nc.vector.tensor_add(it_[:], ei[:], psO[:, 256:512])
                        nc.vector.tensor_sub(ib[:], ei[:], psO[:, 256:512])
                        t2r.append((rt, rb))
                        t2i.append((it_, ib))
                    # stage 4: out = sum_q T2r^T Gw1 + T2i^T Gw2
                    ostg = workp.tile([128, NBLK, W], f32, name="ostg", tag="ostg")
                    for m in range(NBLK):
                        half = m // 2          # 0: n_r in [0,256) -> top
                        ms = slice((m % 2) * 128, (m % 2) * 128 + 128)
                        pso = psump1.tile([128, W], f32, name="ps4", tag="ps4")
                        for qi in range(3):
                            nc.tensor.matmul(pso[:], t2r[qi][half][:, ms],
                                             cst["Gw1"][qi][:],
                                             start=(qi == 0), stop=False)
                        for qi in range(3):
                            nc.tensor.matmul(pso[:], t2i[qi][half][:, ms],
                                             cst["Gw2"][qi][:],
                                             start=False, stop=(qi == 2))
                        nc.scalar.mul(ostg[:, m, :], pso[:], 1.0 / 256.0)
                        if m == 1:
                            nc.sync.dma_start(
                                blocked(out_ext[b, c])[:, 0:2, :], ostg[:, 0:2, :])
                    nc.sync.dma_start(
                        blocked(out_ext[b, c])[:, 2:4, :], ostg[:, 2:4, :])


            # Interleave: keep each collective's consumers ~one channel of
            # work downstream in every engine stream, so nothing queues
            # behind an in-flight AllReduce (the scheduler otherwise hoists
            # pass-2 ops early and stalls the whole machine ~30us per AR).
            emit_p1(0)
            load_consts([n for n in cdefs if n not in p1_consts])
            emit_p1(1)
            emit_p1(2)
            emit_p2(0)
            emit_p2(1)
            emit_p2(2)
    nc.compile()
    return nc


def _host_inputs(x, running_amp):
    j = np.arange(H)
    theta = -2.0 * np.pi * np.outer(j, j) / H
    Fc = np.exp(1j * theta)           # F[n,k] = w^{nk}
    Gc = np.conj(Fc)                  # G[n,k] = w^{-nk}

    def halfpad(M):
        out = np.zeros((H, KC), np.float32)
        out[:, :257] = M[:, :257]
        return out

    # stage1 radix-2: B0/B1 = even/odd columns of F, top 256 rows
    B0 = Fc[0:256, 0::2]
    B1 = Fc[0:256, 1::2]
    # stage3 radix-2 DIT: even/odd columns of G restricted per derivation
    n256 = np.arange(256)
    Me = np.exp(2j * np.pi * np.outer(n256, n256) / 256.0)      # G[n,2k'] on n<256
    Mo = np.exp(2j * np.pi * np.outer(2 * n256 + 1, n256) / 512.0)  # [k',n']
    MeRI = np.concatenate([Me.real, Me.imag], axis=1)
    MeIR = np.concatenate([-Me.imag, Me.real], axis=1)
    MoRI = np.concatenate([Mo.real, Mo.imag], axis=1)
    MoIR = np.concatenate([-Mo.imag, Mo.real], axis=1)

    # stage-4 weights: w_k in {1,2}, zero on pad; scaled by 1/N^2
    wgt = np.zeros(KC)
    wgt[0] = 1.0
    wgt[1:256] = 2.0
    wgt[256] = 1.0
    Gw1 = np.zeros((KC, W), np.float32)
    Gw2 = np.zeros((KC, W), np.float32)
    Gw1[:257] = (wgt[:257, None] * Gc[:257, :].real * 256.0 / (H * W)).astype(np.float32)
    Gw2[:257] = (-wgt[:257, None] * Gc[:257, :].imag * 256.0 / (H * W)).astype(np.float32)

    f32 = np.float32
    consts = {
        "B0r": B0.real.astype(f32), "B0i": B0.imag.astype(f32),
        "B1r": B1.real.astype(f32), "B1i": B1.imag.astype(f32),
        "Frh": halfpad(Fc.real.astype(f32)), "Fih": halfpad(Fc.imag.astype(f32)),
        "negFih": halfpad((-Fc.imag).astype(f32)),
        "MeRI": MeRI.astype(f32), "MeIR": MeIR.astype(f32),
        "MoRI": MoRI.astype(f32), "MoIR": MoIR.astype(f32),
        "Gw1": Gw1, "Gw2": Gw2,
    }
    nrows = sum(a.shape[0] for a in consts.values()) + 1
    cpack = np.zeros((nrows, 512), f32)
    r = 0
    for a in consts.values():
        cpack[r:r + a.shape[0], :a.shape[1]] = a
        r += a.shape[0]

    perm_kr = np.concatenate([np.arange(0, H, 2), np.arange(1, H, 2)])
    if abs(float(running_amp.sum())) == 0.0:
        ra_half = np.zeros((C, H, KC), np.float32)
        mom_eff = 1.0 / B
    else:
        ra_s = np.fft.ifftshift(running_amp, axes=(-2, -1)).astype(np.float64)
        ra_rev = ra_s[:, (-np.arange(H)) % H][:, :, (-np.arange(W)) % W]
        ra_sym = (1.0 - MOMENTUM) * 0.5 * (ra_s + ra_rev)
        ra_half = np.zeros((C, H, KC), np.float32)
        ra_half[:, :, :257] = ra_sym[:, perm_kr][:, :, :257].astype(np.float32)
        mom_eff = MOMENTUM / B
    cpack[r, 0:128] = mom_eff

    in_maps = []
    for i in range(N_CORES):
        m = {"x": np.ascontiguousarray(x[i * B_LOC:(i + 1) * B_LOC]),
             "ra": ra_half, "cpack": cpack}
        in_maps.append(m)
    return in_maps


def kernel(x: np.ndarray, running_amp: np.ndarray) -> np.ndarray:
    from concourse.bass_utils import run_bass_kernel_spmd

    if "nc" not in _cached:
        _cached["nc"] = _build()
    nc = _cached["nc"]
    in_maps = _host_inputs(np.asarray(x, np.float32),
                           np.asarray(running_amp, np.float32))
    res = run_bass_kernel_spmd(nc, in_maps, list(range(N_CORES)))
    out = np.concatenate([res.results[i]["out"] for i in range(N_CORES)], axis=0)
    return out.astype(np.float32)

